# revision 2
# baseline (speedup 1.0000x reference)
"""AbstractContrastiveLoss on 8 TRN2 NeuronCores.

Data-parallel over (sample, half) -> 8 shards. Gather-free formulation:
all per-pixel class-dependent terms flow through TWO bf16 matmuls against
a host-precomputed one-hot:
  forward : sumsT = e_bf16 @ oh            (16, C)  segment sums
  backward: g_aug = oh @ [mu | sq | w]     (F, 18)  per-pixel gather of
            mu[t], ||mu[t]||^2 and 1/count[t] in ONE matmul.
counts (and 1/counts) are pure functions of the int target and are
precomputed on host, so no on-device one-hot build, no int gathers, and
no fp32 matmuls (fp32r is multi-pass on PE; bf16 is full rate).
Cross-shard reduction of sums via psum pairs; final mean on host.
"""

import numpy as np

N, E, C = 4, 16, 64
H = W = 768
P = H * W
F = P // 2
DELTA_VAR = 0.5
DELTA_DIST = 2.0
ALPHA, BETA, GAMMA = 1.0, 1.0, 0.001
EPS = 1e-12

_CACHE = {}


def _build():
    import jax
    import jax.numpy as jnp
    from jax.sharding import Mesh, PartitionSpec as PS
    from jax.experimental.shard_map import shard_map

    devs = jax.devices()[:8]
    mesh = Mesh(np.asarray(devs), ("c",))

    def shard_fn(e, oh, winv):
        # e:    (1, 16, F) f32   this core's half-sample, feature-major
        # oh:   (1, F, C)  bf16  host-built one-hot of target
        # winv: (1, C)     f32   1/max(counts,1) for this core's sample
        e = e[0]
        oh = oh[0]
        winv = winv[0]

        # forward: per-class segment sums, bf16 matmul, f32 accumulation
        sums = jnp.einsum(
            "jf,fc->cj",
            e.astype(jnp.bfloat16),
            oh,
            preferred_element_type=jnp.float32,
        )  # (C, 16)
        red = jax.lax.psum(
            sums, "c", axis_index_groups=[[0, 1], [2, 3], [4, 5], [6, 7]]
        )
        mu = red * winv[:, None]  # (C, 16)

        # backward: fused gather of [mu | sq | w] via one bf16 matmul
        sq = jnp.sum(mu * mu, axis=1)  # (C,)
        M = jnp.concatenate(
            [mu, sq[:, None], winv[:, None]], axis=1
        )  # (C, 18)
        g_aug = jnp.einsum(
            "fc,cj->fj", oh, M.astype(jnp.bfloat16),
            preferred_element_type=jnp.float32,
        )  # (F, 18)
        g = g_aug[:, :E]          # mu[t]     (F, 16)
        sq_t = g_aug[:, E]        # ||mu[t]||^2
        w_t = g_aug[:, E + 1]     # 1/count[t]

        # d^2 = ||e||^2 - 2 e.mu[t] + ||mu[t]||^2
        en2 = jnp.sum(e * e, axis=0)          # (F,)
        dot = jnp.einsum("jf,fj->f", e, g)    # (F,)
        d2 = jnp.maximum(en2 - 2.0 * dot + sq_t, 0.0)
        d = jnp.sqrt(d2 + EPS)
        h = jnp.maximum(d - DELTA_VAR, 0.0)
        var_part = jnp.sum(h * h * w_t)

        # distance + regularizer terms on (C, C): tiny, f32
        gram = mu @ mu.T
        dmat = jnp.sqrt(
            jnp.maximum(sq[:, None] + sq[None, :] - 2 * gram, 0.0) + EPS
        )
        rep = 2.0 * DELTA_DIST * (1.0 - jnp.eye(C, dtype=e.dtype))
        dist = jnp.sum(jnp.maximum(rep - dmat, 0.0) ** 2)
        reg = jnp.sum(jnp.sqrt(sq + EPS))
        out = jnp.stack([var_part, dist, reg])
        return out[None, :]  # (1, 3)

    fn = shard_map(
        shard_fn, mesh=mesh,
        in_specs=(PS("c"), PS("c"), PS("c")),
        out_specs=PS("c"),
        check_rep=False,
    )
    return jax.jit(fn)


def _prep(input_, target):
    """Host-side shard + one-hot precompute (pure layout/int work)."""
    import jax.numpy as jnp

    e8 = np.stack(
        [
            input_[c // 2].reshape(E, P)[:, (c % 2) * F: (c % 2 + 1) * F]
            for c in range(8)
        ]
    )
    t8 = np.stack(
        [
            target[c // 2].reshape(P)[(c % 2) * F: (c % 2 + 1) * F]
            for c in range(8)
        ]
    )
    # one-hot in bf16 (exact 0/1)
    oh8 = (t8[:, :, None] == np.arange(C, dtype=np.int32)[None, None, :])
    oh8 = jnp.asarray(oh8).astype(jnp.bfloat16)
    # per-sample counts -> per-core 1/max(counts,1)
    w8 = np.empty((8, C), dtype=np.float32)
    for n in range(N):
        cnt = np.bincount(target[n].reshape(-1), minlength=C).astype(np.float32)
        winv = 1.0 / np.maximum(cnt, 1.0)
        w8[2 * n] = winv
        w8[2 * n + 1] = winv
    return e8, oh8, w8


def kernel(input_, target):
    import jax.numpy as jnp

    input_ = np.ascontiguousarray(np.asarray(input_, dtype=np.float32))
    target = np.ascontiguousarray(np.asarray(target, dtype=np.int32))

    if "fn" not in _CACHE:
        _CACHE["fn"] = _build()
    fn = _CACHE["fn"]

    e8, oh8, w8 = _prep(input_, target)
    out = np.asarray(fn(jnp.asarray(e8), oh8, jnp.asarray(w8)))  # (8, 3)

    loss = 0.0
    for n in range(N):
        a, b = out[2 * n], out[2 * n + 1]
        var = (float(a[0]) + float(b[0])) / C
        dist = float(a[1]) / (C * (C - 1))
        reg = float(a[2]) / C
        loss += ALPHA * var + BETA * dist + GAMMA * reg
    return np.float32(loss / N)


# revision 3
# speedup vs baseline: 1.8938x; 1.8938x over previous
"""AbstractContrastiveLoss on 8 TRN2 NeuronCores.

Data-parallel over (sample, half) -> 8 shards. Gather-free formulation:
all per-pixel class-dependent terms flow through TWO bf16 matmuls against
a host-precomputed one-hot, with e shipped PRE-TRANSPOSED (F, 16) so both
matmuls contract/produce along the pixel-major axis and the distance is a
pure free-axis reduce:
  forward : sums  = oh^T @ e_bf16          (C, 16)  segment sums
  backward: g_aug = oh @ [mu | w]          (F, 17)  per-pixel gather of
            mu[t] and 1/count[t] in ONE matmul.
  d^2     = sum((eT - g)^2, axis=-1)       free-axis reduce, no transpose,
            no batched per-pixel dot, no cross-partition reduction.
counts (and 1/counts) are pure functions of the int target and are
precomputed on host, so no on-device one-hot build and no int gathers.
Cross-shard reduction of sums via psum pairs; final mean on host.
"""

import numpy as np

N, E, C = 4, 16, 64
H = W = 768
P = H * W
F = P // 2
DELTA_VAR = 0.5
DELTA_DIST = 2.0
ALPHA, BETA, GAMMA = 1.0, 1.0, 0.001
EPS = 1e-12

_CACHE = {}


def _build():
    import jax
    import jax.numpy as jnp
    from jax.sharding import Mesh, PartitionSpec as PS
    from jax.experimental.shard_map import shard_map

    devs = jax.devices()[:8]
    mesh = Mesh(np.asarray(devs), ("c",))

    def shard_fn(et, oh, winv):
        # et:   (1, F, 16) f32   this core's half-sample, PIXEL-major
        # oh:   (1, F, C)  bf16  host-built one-hot of target
        # winv: (1, C)     f32   1/max(counts,1) for this core's sample
        et = et[0]
        oh = oh[0]
        winv = winv[0]

        # forward: per-class segment sums; contraction over pixels, both
        # operands pixel-major -> clean PE lowering, f32 accumulation.
        sums = jnp.einsum(
            "fc,fj->cj",
            oh,
            et.astype(jnp.bfloat16),
            preferred_element_type=jnp.float32,
        )  # (C, 16)
        red = jax.lax.psum(
            sums, "c", axis_index_groups=[[0, 1], [2, 3], [4, 5], [6, 7]]
        )
        mu = red * winv[:, None]  # (C, 16)

        # backward: fused gather of [mu | w] via one bf16 matmul, output
        # pixel-major (F, 17) in the SAME layout as et.
        M = jnp.concatenate([mu, winv[:, None]], axis=1)  # (C, 17)
        g_aug = jnp.einsum(
            "fc,cj->fj", oh, M.astype(jnp.bfloat16),
            preferred_element_type=jnp.float32,
        )  # (F, 17)
        g = g_aug[:, :E]      # mu[t]      (F, 16)
        w_t = g_aug[:, E]     # 1/count[t] (F,)

        # d^2 via direct difference: free-axis reduce, no transposes.
        diff = et - g
        d2 = jnp.sum(diff * diff, axis=1)  # (F,)
        d = jnp.sqrt(d2 + EPS)
        h = jnp.maximum(d - DELTA_VAR, 0.0)
        var_part = jnp.sum(h * h * w_t)

        # distance + regularizer terms on (C, C): tiny, f32
        sq = jnp.sum(mu * mu, axis=1)  # (C,)
        gram = mu @ mu.T
        dmat = jnp.sqrt(
            jnp.maximum(sq[:, None] + sq[None, :] - 2 * gram, 0.0) + EPS
        )
        rep = 2.0 * DELTA_DIST * (1.0 - jnp.eye(C, dtype=et.dtype))
        dist = jnp.sum(jnp.maximum(rep - dmat, 0.0) ** 2)
        reg = jnp.sum(jnp.sqrt(sq + EPS))
        out = jnp.stack([var_part, dist, reg])
        return out[None, :]  # (1, 3)

    fn = shard_map(
        shard_fn, mesh=mesh,
        in_specs=(PS("c"), PS("c"), PS("c")),
        out_specs=PS("c"),
        check_rep=False,
    )
    return jax.jit(fn)


def _prep(input_, target):
    """Host-side shard + transpose + one-hot precompute (layout/int work)."""
    import jax.numpy as jnp

    # (8, F, 16) pixel-major shards
    e8 = np.stack(
        [
            np.ascontiguousarray(
                input_[c // 2]
                .reshape(E, P)[:, (c % 2) * F: (c % 2 + 1) * F]
                .T
            )
            for c in range(8)
        ]
    )
    t8 = np.stack(
        [
            target[c // 2].reshape(P)[(c % 2) * F: (c % 2 + 1) * F]
            for c in range(8)
        ]
    )
    # one-hot in bf16 (exact 0/1)
    oh8 = (t8[:, :, None] == np.arange(C, dtype=np.int32)[None, None, :])
    oh8 = jnp.asarray(oh8).astype(jnp.bfloat16)
    # per-sample counts -> per-core 1/max(counts,1)
    w8 = np.empty((8, C), dtype=np.float32)
    for n in range(N):
        cnt = np.bincount(target[n].reshape(-1), minlength=C).astype(np.float32)
        winv = 1.0 / np.maximum(cnt, 1.0)
        w8[2 * n] = winv
        w8[2 * n + 1] = winv
    return e8, oh8, w8


def kernel(input_, target):
    import jax.numpy as jnp

    input_ = np.ascontiguousarray(np.asarray(input_, dtype=np.float32))
    target = np.ascontiguousarray(np.asarray(target, dtype=np.int32))

    if "fn" not in _CACHE:
        _CACHE["fn"] = _build()
    fn = _CACHE["fn"]

    e8, oh8, w8 = _prep(input_, target)
    out = np.asarray(fn(jnp.asarray(e8), oh8, jnp.asarray(w8)))  # (8, 3)

    loss = 0.0
    for n in range(N):
        a, b = out[2 * n], out[2 * n + 1]
        var = (float(a[0]) + float(b[0])) / C
        dist = float(a[1]) / (C * (C - 1))
        reg = float(a[2]) / C
        loss += ALPHA * var + BETA * dist + GAMMA * reg
    return np.float32(loss / N)
